# revision 1
# baseline (speedup 1.0000x reference)
"""Trainium2 Bass kernel for a 3-layer GCN encoder (PyG GCNConv x3 + global mean pool).

Strategy (8 NeuronCores):
  - Nodes are sharded contiguously across the 8 cores (6250 nodes each); edges
    (with self-loops appended) are partitioned by destination node, bucketed per
    128-node destination block, and split by source half (int16 gather-index
    limit), all on the host.
  - Per layer:  out = A_hat @ (h @ W) + b  is reassociated as
    (A_hat @ h) @ W + b.  Each core computes, for its destination shard,
        agg[d] = sum_{e->d} ew_e * g[src_e]        (g = dinv ⊙ h, self edges ew=1)
        h'[d]  = relu(dinv[d] * (agg @ W)[d] + b)
    The per-edge gather of g rows (512 B each) uses dma_gather from HBM; the
    segment-sum uses a per-chunk weighted one-hot (built on DVE from iota +
    slot ids) contracted on the TensorEngine into PSUM.
  - g for the next layer is produced per-shard and AllGathered (the halo
    exchange degenerates to an AllGather for a uniform random graph).
  - Degrees (deg = segsum(ew, dst) + 1) are computed on-device with the same
    one-hot machinery; dinv = 1/sqrt(deg).
  - Final global mean pool: per-core partial sums via a one-hot matmul; the 8
    [64,128] partials are summed and divided by counts on the host (unshard).
"""

import os
import sys

import numpy as np

for _p in ("/opt/trn_rl_repo",):
    if _p not in sys.path and os.path.isdir(_p):
        sys.path.insert(0, _p)

import concourse.bass as bass
import concourse.bacc as bacc
import concourse.tile as tile
import concourse.mybir as mybir
from concourse import bass_utils
from concourse.alu_op_type import AluOpType

F32 = mybir.dt.float32
I16 = mybir.dt.int16
AF = mybir.ActivationFunctionType
GATHER_MAX_CHUNK = 8  # 8 chunks = 1024 idxs per dma_gather (ring capacity)


class Cfg:
    def __init__(self, n_nodes=50000, n_cores=8, d=128, n_graphs=64, split=32768):
        assert n_nodes % n_cores == 0
        self.n_nodes = n_nodes
        self.n_cores = n_cores
        self.d = d
        self.n_graphs = n_graphs
        self.split = split  # gather-table split point (int16 index limit)
        self.shard = n_nodes // n_cores
        self.blk = 128
        self.n_blk = (self.shard + 127) // 128

    def slots(self, b):
        return min(128, self.shard - b * 128)


def preprocess(cfg, edge_index, edge_weight):
    """Host-side edge bucketing. Returns per-core gather/one-hot input arrays
    plus the (core-uniform) per-(block,half) padded segment sizes."""
    src = np.asarray(edge_index)[0].astype(np.int64)
    dst = np.asarray(edge_index)[1].astype(np.int64)
    ew = np.asarray(edge_weight).astype(np.float32)
    n = cfg.n_nodes
    loop = np.arange(n, dtype=np.int64)
    src = np.concatenate([src, loop])
    dst = np.concatenate([dst, loop])
    ew = np.concatenate([ew, np.ones(n, np.float32)])
    ne = src.shape[0]

    core = dst // cfg.shard
    loc = dst - core * cfg.shard
    blkid = loc // 128
    slot = (loc - blkid * 128).astype(np.float32)
    half = (src >= cfg.split).astype(np.int64)
    key = (core * cfg.n_blk + blkid) * 2 + half

    order = np.argsort(key, kind="stable")
    nkeys = cfg.n_cores * cfg.n_blk * 2
    cnt = np.bincount(key, minlength=nkeys).reshape(cfg.n_cores, cfg.n_blk * 2)
    # per-(block,half) chunk-padded sizes, shared across cores (SPMD program)
    seg = ((cnt.max(axis=0) + 127) // 128) * 128  # [n_blk*2]
    seg_off = np.concatenate([[0], np.cumsum(seg)])  # [n_blk*2+1]
    ep = int(seg_off[-1])

    # position of each edge inside its padded per-core segment
    cnt_flat = np.bincount(key, minlength=nkeys)
    starts = np.concatenate([[0], np.cumsum(cnt_flat)])[:-1]
    sk = key[order]
    rank = np.arange(ne) - starts[sk]
    bh = sk % (cfg.n_blk * 2)
    pos = seg_off[bh] + rank
    core_s = sk // (cfg.n_blk * 2)

    idx16 = np.zeros((cfg.n_cores, ep), np.int16)
    slotf = np.zeros((cfg.n_cores, ep), np.float32)
    ewf = np.zeros((cfg.n_cores, ep), np.float32)
    idx16[core_s, pos] = (src[order] - half[order] * cfg.split).astype(np.int16)
    slotf[core_s, pos] = slot[order]
    ewf[core_s, pos] = ew[order]

    # wrapped index layout: edge i -> [i%16, i//16], replicated to 128 partitions
    idxw = idx16.reshape(cfg.n_cores, ep // 16, 16).transpose(0, 2, 1)
    idxw = np.tile(idxw, (1, 8, 1)).copy()  # [cores, 128, ep//16]
    # chunk layout for DVE scalars: edge i -> [i%128, i//128]
    slotw = slotf.reshape(cfg.n_cores, ep // 128, 128).transpose(0, 2, 1).copy()
    eww = ewf.reshape(cfg.n_cores, ep // 128, 128).transpose(0, 2, 1).copy()

    return dict(seg=seg.reshape(cfg.n_blk, 2), seg_off=seg_off, ep=ep,
                idxw=idxw, slotw=slotw, eww=eww)


def build_program(cfg, seg, seg_off, ep, trunc=""):
    """Build the SPMD Bass/Tile program. Trip counts depend only on seg/ep.

    trunc: debug knob — "B" stops after the degree/g0 phase, "AG" after the
    first AllGather, "L0"/"L1" after layer 0/1 (outputs are then garbage)."""
    n_blk, d, g64 = cfg.n_blk, cfg.d, cfg.n_graphs
    # SWDGE descriptor-ring capacity is ~1024+16 descriptors per queue; a
    # single dma_gather needs ~num_idxs+16, so calls are capped at
    # GATHER_MAX_IDX and spread round-robin over the SWDGE queues.
    n_queues = 2
    nc = bacc.Bacc("TRN2", target_bir_lowering=False, debug=False,
                   enable_asserts=False, num_devices=cfg.n_cores,
                   num_swdge_queues=n_queues)

    x_in = nc.dram_tensor("x_shard", [cfg.shard, d], F32, kind="ExternalInput")
    idx_in = nc.dram_tensor("idxw", [128, ep // 16], I16, kind="ExternalInput")
    slot_in = nc.dram_tensor("slotw", [128, ep // 128], F32, kind="ExternalInput")
    ew_in = nc.dram_tensor("eww", [128, ep // 128], F32, kind="ExternalInput")
    pool_in = nc.dram_tensor("poolm", [128, n_blk * g64], F32, kind="ExternalInput")
    iota_in = nc.dram_tensor("iota", [128, 128], F32, kind="ExternalInput")
    w_in = nc.dram_tensor("wmats", [3, d, d], F32, kind="ExternalInput")
    b_in = nc.dram_tensor("biasb", [3, 128, d], F32, kind="ExternalInput")
    out_t = nc.dram_tensor("pool_out", [g64, d], F32, kind="ExternalOutput")

    g_loc = [nc.dram_tensor(f"g_loc{k}", [cfg.shard, d], F32, kind="Internal")
             for k in range(3)]
    g_full = [nc.dram_tensor(f"g_full{k}", [cfg.n_nodes, d], F32,
                             kind="Internal", addr_space="Shared")
              for k in range(3)]
    rg = [list(range(cfg.n_cores))]

    def block_cols(b):
        """(half, chunk-col, first-in-block, last-in-block) for block b."""
        cols = []
        for h in (0, 1):
            n_ch = int(seg[b, h]) // 128
            c0 = int(seg_off[b * 2 + h]) // 128
            for i in range(n_ch):
                cols.append((h, c0 + i))
        return cols

    with tile.TileContext(nc) as tc:
        with tc.tile_pool(name="const", bufs=1) as cp:
            iota_sb = cp.tile([128, 128], F32, tag="iota")
            nc.sync.dma_start(iota_sb[:, :], iota_in.ap())
            ones_sb = cp.tile([128, 1], F32, tag="ones")
            nc.vector.memset(ones_sb[:, :], 1.0)
            w_sb = []
            b_sb = []
            for k in range(3):
                wt = cp.tile([d, d], F32, tag=f"w{k}", name=f"w{k}")
                nc.sync.dma_start(wt[:, :], w_in.ap()[k, :, :])
                w_sb.append(wt)
                bt = cp.tile([128, d], F32, tag=f"b{k}", name=f"b{k}")
                nc.sync.dma_start(bt[:, :], b_in.ap()[k, :, :])
                b_sb.append(bt)
            poolm_sb = cp.tile([128, n_blk * g64], F32, tag="poolm")
            nc.sync.dma_start(poolm_sb[:, :], pool_in.ap())
            idx_sb = cp.tile([128, ep // 16], I16, tag="idx")
            nc.sync.dma_start(idx_sb[:, :], idx_in.ap())
            slot_sb = cp.tile([128, ep // 128], F32, tag="slot")
            nc.sync.dma_start(slot_sb[:, :], slot_in.ap())
            ew_sb = cp.tile([128, ep // 128], F32, tag="ew")
            nc.sync.dma_start(ew_sb[:, :], ew_in.ap())
            dinv_sb = cp.tile([128, n_blk], F32, tag="dinv")
            pool_acc = cp.tile([g64, d], F32, tag="pacc")
            nc.vector.memset(pool_acc[:, :], 0.0)

            # ---- Phase B: degrees -> dinv -> g0 = dinv * x ----
            blvl = 99 if not trunc.startswith("B") or trunc == "B" else int(trunc[1:])
            with tc.tile_pool(name="degp", bufs=2, space="PSUM") as psD, \
                 tc.tile_pool(name="ohB", bufs=4) as ohpB, \
                 tc.tile_pool(name="workB", bufs=3) as wpB:
                for b in range(n_blk):
                    s = cfg.slots(b)
                    cols = block_cols(b)
                    if blvl >= 1:
                        pd = psD.tile([s, 1], F32, tag="deg", name=f"deg{b}")
                        for j, (_h, col) in enumerate(cols):
                            if blvl < 2 and j > 0:
                                continue
                            oh = ohpB.tile([128, s], F32, tag="oh",
                                           name=f"dg_oh{b}_{j}")
                            nc.vector.tensor_scalar(
                                oh[:, :], iota_sb[:, :s],
                                slot_sb[:, col:col + 1], ew_sb[:, col:col + 1],
                                AluOpType.is_equal, AluOpType.mult)
                            if blvl >= 2:
                                nc.tensor.matmul(
                                    pd[:, :], oh[:, :], ones_sb[:, :],
                                    start=(j == 0), stop=(j == len(cols) - 1))
                    if blvl >= 3:
                        srt = wpB.tile([s, 1], F32, tag="srt", name=f"srt{b}")
                        nc.scalar.sqrt(srt[:, :], pd[:, :])
                        nc.vector.reciprocal(dinv_sb[:s, b:b + 1], srt[:, :])
                    if blvl >= 4:
                        xt = wpB.tile([s, d], F32, tag="xt", name=f"xt{b}")
                        nc.sync.dma_start(xt[:, :],
                                          x_in.ap()[b * 128:b * 128 + s, :])
                        gt = wpB.tile([s, d], F32, tag="gt", name=f"gt{b}")
                        nc.vector.tensor_scalar(gt[:, :], xt[:, :],
                                                dinv_sb[:s, b:b + 1], None,
                                                AluOpType.mult)
                        nc.sync.dma_start(
                            g_loc[0].ap()[b * 128:b * 128 + s, :], gt[:, :])

            if not trunc.startswith("B"):
                nc.gpsimd.collective_compute(
                    "AllGather", AluOpType.bypass, replica_groups=rg,
                    ins=[g_loc[0].ap()], outs=[g_full[0].ap()])

            # ---- Phase C: the three GCN layers ----
            with tc.tile_pool(name="aggp", bufs=2, space="PSUM") as psA, \
                 tc.tile_pool(name="outp", bufs=2, space="PSUM") as psB, \
                 tc.tile_pool(name="poolp", bufs=2, space="PSUM") as psC, \
                 tc.tile_pool(name="ohC", bufs=4) as ohp, \
                 tc.tile_pool(name="stage", bufs=2) as stp, \
                 tc.tile_pool(name="workC", bufs=3) as wp:
                gq = [0]  # round-robin gather queue counter
                if trunc.startswith("B") or trunc == "AG":
                    n_layers = 0
                else:
                    n_layers = {"L0": 1, "L1": 2}.get(trunc, 3)
                max_blk = n_blk
                stop_at = 99  # 1: gathers only, 2: +onehot/agg, 3: +finalize
                if trunc.startswith("G"):
                    n_layers = 1
                    stop_at = 1 if trunc == "G" else 2
                elif trunc.startswith("NB"):
                    n_layers = 1
                    max_blk = int(trunc[2:])
                for k in range(n_layers):
                    gsrc = g_full[k].ap()
                    for b in range(min(n_blk, max_blk)):
                        s = cfg.slots(b)
                        stg = {}
                        for h in (0, 1):
                            n_ch = int(seg[b, h]) // 128
                            if n_ch == 0:
                                continue
                            o16 = int(seg_off[b * 2 + h]) // 16
                            st = stp.tile([128, n_ch, 128], F32, tag=f"st{h}",
                                          name=f"st{k}_{b}_{h}")
                            base = (gsrc[0:cfg.split, :] if h == 0
                                    else gsrc[cfg.split:cfg.n_nodes, :])
                            for c0 in range(0, n_ch, GATHER_MAX_CHUNK):
                                c1 = min(c0 + GATHER_MAX_CHUNK, n_ch)
                                nidx = (c1 - c0) * 128
                                so16 = o16 + c0 * 8
                                nc.gpsimd.dma_gather(
                                    st[:, c0:c1, :], base,
                                    idx_sb[:, so16:so16 + nidx // 16],
                                    nidx, nidx, d,
                                    queue_num=gq[0] % n_queues)
                                gq[0] += 1
                            stg[h] = st
                        if stop_at < 2:
                            continue
                        cols = block_cols(b)
                        pagg = psA.tile([128, s], F32, tag="agg",
                                        name=f"agg{k}_{b}")
                        for j, (h, col) in enumerate(cols):
                            i = col - int(seg_off[b * 2 + h]) // 128
                            oh = ohp.tile([128, s], F32, tag="oh",
                                          name=f"oh{k}_{b}_{j}")
                            nc.vector.tensor_scalar(
                                oh[:, :], iota_sb[:, :s],
                                slot_sb[:, col:col + 1], ew_sb[:, col:col + 1],
                                AluOpType.is_equal, AluOpType.mult)
                            nc.tensor.matmul(pagg[:, :], stg[h][:, i, :],
                                             oh[:, :], start=(j == 0),
                                             stop=(j == len(cols) - 1))
                        if stop_at < 3:
                            continue
                        aggT = wp.tile([128, s], F32, tag="aggT",
                                       name=f"aggT{k}_{b}")
                        nc.scalar.copy(aggT[:, :], pagg[:, :])
                        pout = psB.tile([s, d], F32, tag="out",
                                        name=f"out{k}_{b}")
                        nc.tensor.matmul(pout[:, :], aggT[:, :], w_sb[k][:, :],
                                         start=True, stop=True)
                        t2 = wp.tile([s, d], F32, tag="t2", name=f"t2{k}_{b}")
                        nc.vector.scalar_tensor_tensor(
                            t2[:, :], pout[:, :], dinv_sb[:s, b:b + 1],
                            b_sb[k][:s, :], AluOpType.mult, AluOpType.add)
                        if k < 2:
                            ht = wp.tile([s, d], F32, tag="ht",
                                         name=f"ht{k}_{b}")
                            nc.scalar.activation(ht[:, :], t2[:, :], AF.Relu)
                            gt2 = wp.tile([s, d], F32, tag="gt2",
                                          name=f"gt2{k}_{b}")
                            nc.vector.tensor_scalar(gt2[:, :], ht[:, :],
                                                    dinv_sb[:s, b:b + 1], None,
                                                    AluOpType.mult)
                            nc.sync.dma_start(
                                g_loc[k + 1].ap()[b * 128:b * 128 + s, :],
                                gt2[:, :])
                        else:
                            pp = psC.tile([g64, d], F32, tag="pp",
                                          name=f"pp{b}")
                            nc.tensor.matmul(
                                pp[:, :],
                                poolm_sb[:s, b * g64:(b + 1) * g64],
                                t2[:, :], start=True, stop=True)
                            nc.vector.tensor_tensor(pool_acc[:, :],
                                                    pool_acc[:, :], pp[:, :],
                                                    AluOpType.add)
                    if k < 2:
                        nc.gpsimd.collective_compute(
                            "AllGather", AluOpType.bypass, replica_groups=rg,
                            ins=[g_loc[k + 1].ap()], outs=[g_full[k + 1].ap()])

            nc.sync.dma_start(out_t.ap(), pool_acc[:, :])

    nc.compile()
    return nc


def make_in_maps(cfg, prep, x, batch, ws, bs):
    x = np.ascontiguousarray(np.asarray(x, np.float32))
    batch = np.asarray(batch).astype(np.int64)
    wmats = np.stack([np.asarray(w, np.float32) for w in ws])
    biasb = np.stack([np.broadcast_to(np.asarray(b, np.float32), (128, cfg.d))
                      for b in bs]).copy()
    iota = np.tile(np.arange(128, dtype=np.float32), (128, 1)).copy()

    # pooling one-hot: local node l (block b=l//128, part p=l%128) -> graph id
    poolm = np.zeros((cfg.n_cores, 128, cfg.n_blk * cfg.n_graphs), np.float32)
    c_idx = np.repeat(np.arange(cfg.n_cores), cfg.shard)
    l = np.tile(np.arange(cfg.shard), cfg.n_cores)
    poolm[c_idx, l % 128, (l // 128) * cfg.n_graphs + batch] = 1.0

    in_maps = []
    for c in range(cfg.n_cores):
        in_maps.append({
            "x_shard": x[c * cfg.shard:(c + 1) * cfg.shard],
            "idxw": prep["idxw"][c],
            "slotw": prep["slotw"][c],
            "eww": prep["eww"][c],
            "poolm": poolm[c],
            "iota": iota,
            "wmats": wmats,
            "biasb": biasb,
        })
    counts = np.bincount(batch, minlength=cfg.n_graphs).astype(np.float32)
    return in_maps, counts


_PROGRAM_CACHE = {}


def run(cfg, x, edge_index, edge_weight, batch, ws, bs, trace=False, trunc=""):
    prep = preprocess(cfg, edge_index, edge_weight)
    key = (cfg.n_nodes, cfg.n_cores, cfg.d, cfg.n_graphs, cfg.split,
           prep["ep"], tuple(prep["seg"].ravel()), trunc)
    nc = _PROGRAM_CACHE.get(key)
    if nc is None:
        nc = build_program(cfg, prep["seg"], prep["seg_off"], prep["ep"],
                           trunc=trunc)
        _PROGRAM_CACHE[key] = nc
    in_maps, counts = make_in_maps(cfg, prep, x, batch, ws, bs)
    res = bass_utils.run_bass_kernel_spmd(
        nc, in_maps, core_ids=list(range(cfg.n_cores)), trace=trace)
    partial = np.zeros((cfg.n_graphs, cfg.d), np.float64)
    for c in range(cfg.n_cores):
        partial += res.results[c]["pool_out"].astype(np.float64)
    out = (partial / np.maximum(counts, 1.0)[:, None]).astype(np.float32)
    return out, res


def kernel(x, edge_index, edge_weight, batch, W0, b0, W1, b1, W2, b2):
    cfg = Cfg()
    trace = bool(int(os.environ.get("GCN_TRACE", "0")))
    out, _ = run(cfg, x, edge_index, edge_weight, batch,
                 [W0, W1, W2], [b0, b1, b2], trace=trace)
    return out



# revision 7
# speedup vs baseline: 2.6705x; 2.6705x over previous
"""Trainium2 Bass kernel for a 3-layer GCN encoder (PyG GCNConv x3 + global mean pool).

Strategy (8 NeuronCores):
  - Nodes sharded contiguously across cores (6250 each); edges (+self-loops)
    partitioned by destination, bucketed per 128-node dst block and split by
    source half (int16 gather-index limit), chunk-128 padded, sizes uniform
    across cores (SPMD program).
  - Symmetric normalization is folded on the host: ew'' = ew * dinv[src] *
    dinv[dst] (self-loop weight dinv[n]^2), so the device computes per layer
        agg[dst] = sum_e ew''_e * h[src_e]          (gathered bf16 rows)
        h'       = relu(agg @ W + b)
    with NO degree phase and NO per-layer dinv multiplies. The layer-0 table
    is x itself (bf16, replicated to every core's HBM by the host).
  - Per dst block: per-edge rows come via dma_gather (256B bf16 rows, 4 SWDGE
    queues, 64KB descriptor scratch -> whole-segment calls); the weighted
    one-hot is built in TWO big-tile DVE tensor_tensor ops per (block,half)
    (iota==slot, then *ew) with stride-0 broadcast APs; the TensorEngine
    contracts gathered-rows x one-hot into PSUM (bf16 operands), seeds the
    bias with a K=1 ones x b matmul, and applies W. Relu + PSUM->SBUF copies
    run on the otherwise-idle scalar engine.
  - h' (bf16) is written to the core's table shard and AllGathered between
    layers. Final global mean pool: per-block one-hot matmul accumulated in
    PSUM across all blocks; host sums the 8 [64,128] partials / counts.
"""

import os
import sys

import numpy as np

for _p in ("/opt/trn_rl_repo",):
    if _p not in sys.path and os.path.isdir(_p):
        sys.path.insert(0, _p)

import concourse.bass as bass
import concourse.bacc as bacc
import concourse.tile as tile
import concourse.mybir as mybir
from concourse import bass_utils
from concourse.alu_op_type import AluOpType

F32 = mybir.dt.float32
BF16 = mybir.dt.bfloat16
I16 = mybir.dt.int16
AF = mybir.ActivationFunctionType

# The SWDGE descriptor ring is ~1024+16 entries per queue on HW (ucode-fixed;
# larger dynamic_dma_scratch_size does NOT raise it — 3968-idx calls hang).
DMA_SCRATCH = int(os.environ.get("GCN_SCRATCH", "16384"))
N_QUEUES = int(os.environ.get("GCN_QUEUES", "4"))
CALL_CHUNKS = int(os.environ.get("GCN_CALLCH", "8"))


class Cfg:
    def __init__(self, n_nodes=50000, n_cores=8, d=128, n_graphs=64, split=32768):
        assert n_nodes % n_cores == 0
        self.n_nodes = n_nodes
        self.n_cores = n_cores
        self.d = d
        self.n_graphs = n_graphs
        self.split = split  # gather-table split point (int16 index limit)
        self.shard = n_nodes // n_cores
        self.blk = 128
        self.n_blk = (self.shard + 127) // 128

    def slots(self, b):
        return min(128, self.shard - b * 128)


def preprocess(cfg, edge_index, edge_weight):
    """Host-side: degrees/dinv, normalization folding, edge bucketing.
    Returns per-core gather/one-hot arrays + core-uniform segment sizes."""
    src = np.asarray(edge_index)[0].astype(np.int64)
    dst = np.asarray(edge_index)[1].astype(np.int64)
    ew = np.asarray(edge_weight).astype(np.float64)
    n = cfg.n_nodes
    loop = np.arange(n, dtype=np.int64)
    src = np.concatenate([src, loop])
    dst = np.concatenate([dst, loop])
    ew = np.concatenate([ew, np.ones(n, np.float64)])
    ne = src.shape[0]

    deg = np.bincount(dst, weights=ew, minlength=n)  # >= 1 (self-loop)
    dinv = 1.0 / np.sqrt(deg)
    ew = (ew * dinv[src] * dinv[dst]).astype(np.float32)

    core = dst // cfg.shard
    loc = dst - core * cfg.shard
    blkid = loc // 128
    slot = (loc - blkid * 128).astype(np.float32)
    half = (src >= cfg.split).astype(np.int64)
    key = (core * cfg.n_blk + blkid) * 2 + half

    order = np.argsort(key, kind="stable")
    nkeys = cfg.n_cores * cfg.n_blk * 2
    cnt = np.bincount(key, minlength=nkeys).reshape(cfg.n_cores, cfg.n_blk * 2)
    # per-(block,half) chunk-padded sizes, shared across cores (SPMD program)
    seg = ((cnt.max(axis=0) + 127) // 128) * 128  # [n_blk*2]
    seg_off = np.concatenate([[0], np.cumsum(seg)])  # [n_blk*2+1]
    ep = int(seg_off[-1])

    # position of each edge inside its padded per-core segment
    cnt_flat = np.bincount(key, minlength=nkeys)
    starts = np.concatenate([[0], np.cumsum(cnt_flat)])[:-1]
    sk = key[order]
    rank = np.arange(ne) - starts[sk]
    bh = sk % (cfg.n_blk * 2)
    pos = seg_off[bh] + rank
    core_s = sk // (cfg.n_blk * 2)

    idx16 = np.zeros((cfg.n_cores, ep), np.int16)
    slotf = np.zeros((cfg.n_cores, ep), np.float32)
    ewf = np.zeros((cfg.n_cores, ep), np.float32)
    idx16[core_s, pos] = (src[order] - half[order] * cfg.split).astype(np.int16)
    slotf[core_s, pos] = slot[order]
    ewf[core_s, pos] = ew[order]

    # wrapped index layout: edge i -> [i%16, i//16], replicated to 128 partitions
    idxw = idx16.reshape(cfg.n_cores, ep // 16, 16).transpose(0, 2, 1)
    idxw = np.tile(idxw, (1, 8, 1)).copy()  # [cores, 128, ep//16]
    # chunk layout for the one-hot build: edge i -> [i%128, i//128], bf16
    import ml_dtypes
    bf16 = ml_dtypes.bfloat16
    slotw = np.ascontiguousarray(
        slotf.reshape(cfg.n_cores, ep // 128, 128).transpose(0, 2, 1)
    ).astype(bf16)
    eww = np.ascontiguousarray(
        ewf.reshape(cfg.n_cores, ep // 128, 128).transpose(0, 2, 1)
    ).astype(bf16)

    return dict(seg=seg.reshape(cfg.n_blk, 2), seg_off=seg_off, ep=ep,
                idxw=idxw, slotw=slotw, eww=eww)


def build_program(cfg, seg, seg_off, ep, trunc=""):
    """Build the SPMD Bass/Tile program. Trip counts depend only on seg/ep.

    trunc: debug knob - "G" gathers only, "OH" +one-hot, "L0"/"L1" stop
    after layer 0/1 (outputs garbage)."""
    n_blk, d, g64 = cfg.n_blk, cfg.d, cfg.n_graphs
    nc = bacc.Bacc("TRN2", target_bir_lowering=False, debug=False,
                   enable_asserts=False, num_devices=cfg.n_cores,
                   num_swdge_queues=N_QUEUES,
                   dynamic_dma_scratch_size=DMA_SCRATCH)

    t0_in = nc.dram_tensor("t0", [cfg.n_nodes, d], BF16, kind="ExternalInput")
    idx_in = nc.dram_tensor("idxw", [128, ep // 16], I16, kind="ExternalInput")
    slot_in = nc.dram_tensor("slotw", [128, ep // 128], BF16, kind="ExternalInput")
    ew_in = nc.dram_tensor("eww", [128, ep // 128], BF16, kind="ExternalInput")
    pool_in = nc.dram_tensor("poolm", [128, n_blk * g64], BF16, kind="ExternalInput")
    iota_in = nc.dram_tensor("iota", [128, 128], BF16, kind="ExternalInput")
    w_in = nc.dram_tensor("wmats", [3, d, d], BF16, kind="ExternalInput")
    b_in = nc.dram_tensor("biasb", [3, 1, d], BF16, kind="ExternalInput")
    out_t = nc.dram_tensor("pool_out", [g64, d], F32, kind="ExternalOutput")

    g_loc = [None] + [nc.dram_tensor(f"g_loc{k}", [cfg.shard, d], BF16,
                                     kind="Internal") for k in (1, 2)]
    g_full = [None] + [nc.dram_tensor(f"g_full{k}", [cfg.n_nodes, d], BF16,
                                      kind="Internal", addr_space="Shared")
                       for k in (1, 2)]
    rg = [list(range(cfg.n_cores))]

    n_layers = {"G": 1, "OH": 1, "L0": 1, "L1": 2}.get(trunc, 3)
    stop_at = {"G": 1, "OH": 2}.get(trunc, 99)

    with tile.TileContext(nc) as tc:
        with tc.tile_pool(name="const", bufs=1) as cp:
            iota_sb = cp.tile([128, 128], BF16, tag="iota")
            nc.sync.dma_start(iota_sb[:, :], iota_in.ap())
            ones_sb = cp.tile([1, 128], BF16, tag="ones")
            nc.vector.memset(ones_sb[:, :], 1.0)
            w_sb = []
            b_sb = []
            for k in range(3):
                wt = cp.tile([d, d], BF16, tag=f"w{k}", name=f"w{k}")
                nc.sync.dma_start(wt[:, :], w_in.ap()[k, :, :])
                w_sb.append(wt)
                bt = cp.tile([1, d], BF16, tag=f"b{k}", name=f"b{k}")
                nc.sync.dma_start(bt[:, :], b_in.ap()[k, :, :])
                b_sb.append(bt)
            poolm_sb = cp.tile([128, n_blk * g64], BF16, tag="poolm")
            nc.sync.dma_start(poolm_sb[:, :], pool_in.ap())
            idx_sb = cp.tile([128, ep // 16], I16, tag="idx")
            nc.sync.dma_start(idx_sb[:, :], idx_in.ap())
            slot_sb = cp.tile([128, ep // 128], BF16, tag="slot")
            nc.sync.dma_start(slot_sb[:, :], slot_in.ap())
            ew_sb = cp.tile([128, ep // 128], BF16, tag="ew")
            nc.sync.dma_start(ew_sb[:, :], ew_in.ap())

            with tc.tile_pool(name="aggp", bufs=2, space="PSUM") as psA, \
                 tc.tile_pool(name="outp", bufs=2, space="PSUM") as psB, \
                 tc.tile_pool(name="poolp", bufs=1, space="PSUM") as psC, \
                 tc.tile_pool(name="ohp", bufs=3) as ohp, \
                 tc.tile_pool(name="stage", bufs=3) as stp, \
                 tc.tile_pool(name="workp", bufs=3) as wp:
                pool_acc = psC.tile([g64, d], F32, tag="pacc")
                gq = [0]  # round-robin gather queue counter
                for k in range(n_layers):
                    gsrc = t0_in.ap() if k == 0 else g_full[k].ap()
                    for b in range(n_blk):
                        s = cfg.slots(b)
                        stg = {}
                        ohg = {}
                        for h in (0, 1):
                            n_ch = int(seg[b, h]) // 128
                            if n_ch == 0:
                                continue
                            c0seg = int(seg_off[b * 2 + h]) // 128
                            o16 = int(seg_off[b * 2 + h]) // 16
                            st = stp.tile([128, n_ch, d], BF16, tag=f"st{h}",
                                          name=f"st{k}_{b}_{h}")
                            base = (gsrc[0:cfg.split, :] if h == 0
                                    else gsrc[cfg.split:cfg.n_nodes, :])
                            for c0 in range(0, n_ch, CALL_CHUNKS):
                                c1 = min(c0 + CALL_CHUNKS, n_ch)
                                nidx = (c1 - c0) * 128
                                so16 = o16 + c0 * 8
                                nc.gpsimd.dma_gather(
                                    st[:, c0:c1, :], base,
                                    idx_sb[:, so16:so16 + nidx // 16],
                                    nidx, nidx, d,
                                    queue_num=gq[0] % N_QUEUES)
                                gq[0] += 1
                            stg[h] = st
                            if stop_at < 2:
                                continue
                            # weighted one-hot, two big-tile DVE passes
                            oh = ohp.tile([128, n_ch, 128], BF16, tag=f"oh{h}",
                                          name=f"oh{k}_{b}_{h}")
                            iota_b = (iota_sb[:, :].unsqueeze(1)
                                      .broadcast_to([128, n_ch, 128]))
                            slot_b = (slot_sb[:, c0seg:c0seg + n_ch]
                                      .unsqueeze(2)
                                      .broadcast_to([128, n_ch, 128]))
                            ew_b = (ew_sb[:, c0seg:c0seg + n_ch]
                                    .unsqueeze(2)
                                    .broadcast_to([128, n_ch, 128]))
                            nc.vector.tensor_tensor(
                                oh[:, :, :], iota_b, slot_b,
                                AluOpType.is_equal)
                            nc.vector.tensor_tensor(
                                oh[:, :, :], oh[:, :, :], ew_b,
                                AluOpType.mult)
                            ohg[h] = oh
                        if stop_at < 3:
                            continue
                        pagg = psA.tile([d, 128], F32, tag="agg",
                                        name=f"agg{k}_{b}")
                        cols = [(h, i) for h in (0, 1)
                                for i in range(int(seg[b, h]) // 128)]
                        for j, (h, i) in enumerate(cols):
                            nc.tensor.matmul(pagg[:, :s], stg[h][:, i, :],
                                             ohg[h][:, i, :s],
                                             start=(j == 0),
                                             stop=(j == len(cols) - 1))
                        aggT = wp.tile([d, 128], BF16, tag="aggT",
                                       name=f"aggT{k}_{b}")
                        nc.scalar.copy(aggT[:, :s], pagg[:, :s])
                        pout = psB.tile([128, d], F32, tag="out",
                                        name=f"out{k}_{b}")
                        nc.tensor.matmul(pout[:s, :], ones_sb[:, :s],
                                         b_sb[k][:, :], start=True, stop=False)
                        nc.tensor.matmul(pout[:s, :], aggT[:, :s], w_sb[k][:, :],
                                         start=False, stop=True)
                        if k < 2:
                            ht = wp.tile([128, d], BF16, tag="ht",
                                         name=f"ht{k}_{b}")
                            nc.scalar.activation(ht[:s, :], pout[:s, :], AF.Relu)
                            nc.sync.dma_start(
                                g_loc[k + 1].ap()[b * 128:b * 128 + s, :],
                                ht[:s, :])
                        else:
                            t2 = wp.tile([128, d], BF16, tag="t2",
                                         name=f"t2_{b}")
                            nc.scalar.copy(t2[:s, :], pout[:s, :])
                            nc.tensor.matmul(
                                pool_acc[:, :],
                                poolm_sb[:s, b * g64:(b + 1) * g64],
                                t2[:s, :], start=(b == 0),
                                stop=(b == n_blk - 1))
                    if k < 2 and stop_at >= 3:
                        nc.gpsimd.collective_compute(
                            "AllGather", AluOpType.bypass, replica_groups=rg,
                            ins=[g_loc[k + 1].ap()], outs=[g_full[k + 1].ap()])

                if n_layers == 3 and stop_at >= 3:
                    pf = wp.tile([g64, d], F32, tag="pf", name="poolf")
                    nc.scalar.copy(pf[:, :], pool_acc[:, :])
                    nc.sync.dma_start(out_t.ap(), pf[:, :])

    nc.compile()
    return nc


def make_in_maps(cfg, prep, x, batch, ws, bs):
    import ml_dtypes
    bf16 = ml_dtypes.bfloat16
    x16 = np.ascontiguousarray(np.asarray(x, np.float32)).astype(bf16)
    batch = np.asarray(batch).astype(np.int64)
    wmats = np.stack([np.asarray(w, np.float32) for w in ws]).astype(bf16)
    biasb = np.stack([np.asarray(b, np.float32).reshape(1, cfg.d)
                      for b in bs]).astype(bf16)
    iota = np.tile(np.arange(128, dtype=np.float32), (128, 1)).astype(bf16)

    # pooling one-hot: local node l (block b=l//128, part p=l%128) -> graph id
    poolm = np.zeros((cfg.n_cores, 128, cfg.n_blk * cfg.n_graphs), np.float32)
    c_idx = np.repeat(np.arange(cfg.n_cores), cfg.shard)
    l = np.tile(np.arange(cfg.shard), cfg.n_cores)
    poolm[c_idx, l % 128, (l // 128) * cfg.n_graphs + batch] = 1.0
    poolm16 = poolm.astype(bf16)

    in_maps = []
    for c in range(cfg.n_cores):
        in_maps.append({
            "t0": x16,
            "idxw": prep["idxw"][c],
            "slotw": prep["slotw"][c],
            "eww": prep["eww"][c],
            "poolm": poolm16[c],
            "iota": iota,
            "wmats": wmats,
            "biasb": biasb,
        })
    counts = np.bincount(batch, minlength=cfg.n_graphs).astype(np.float32)
    return in_maps, counts


_PROGRAM_CACHE = {}


def run(cfg, x, edge_index, edge_weight, batch, ws, bs, trace=False, trunc=""):
    prep = preprocess(cfg, edge_index, edge_weight)
    key = (cfg.n_nodes, cfg.n_cores, cfg.d, cfg.n_graphs, cfg.split,
           prep["ep"], tuple(prep["seg"].ravel()), trunc)
    nc = _PROGRAM_CACHE.get(key)
    if nc is None:
        nc = build_program(cfg, prep["seg"], prep["seg_off"], prep["ep"],
                           trunc=trunc)
        _PROGRAM_CACHE[key] = nc
    in_maps, counts = make_in_maps(cfg, prep, x, batch, ws, bs)
    res = bass_utils.run_bass_kernel_spmd(
        nc, in_maps, core_ids=list(range(cfg.n_cores)), trace=trace)
    if trunc:
        return np.zeros((cfg.n_graphs, cfg.d), np.float32), res
    partial = np.zeros((cfg.n_graphs, cfg.d), np.float64)
    for c in range(cfg.n_cores):
        partial += res.results[c]["pool_out"].astype(np.float64)
    out = (partial / np.maximum(counts, 1.0)[:, None]).astype(np.float32)
    return out, res


def kernel(x, edge_index, edge_weight, batch, W0, b0, W1, b1, W2, b2):
    cfg = Cfg()
    trace = bool(int(os.environ.get("GCN_TRACE", "0")))
    out, _ = run(cfg, x, edge_index, edge_weight, batch,
                 [W0, W1, W2], [b0, b1, b2], trace=trace)
    return out
